# revision 29
# baseline (speedup 1.0000x reference)
"""MoE (dense-act-dense, top-4 of 8 experts) Trainium2 kernel.

Strategy (expert-parallel, host-side dispatch):
  - The forward combine weight is exactly 1.0 (straight-through gate trick in
    the reference), so out[n] = sum_{e in top4(n)} expert_e(x[n]).
  - Host computes the tiny gate matmul + top-4 routing (0.05% of FLOPs) and
    dispatches tokens: core e receives the tokens routed to expert e
    (capacity-padded), plus expert e's weights. This is the sharding step.
  - Each of the 8 cores runs a dense 2-layer MLP (relu between) on its tokens:
      h = relu(w1[e] @ x) ; y = w2[e] @ h
    as two chained GEMMs in bf16 (fp32 PSUM accumulate).
  - Host scatter-adds per-expert outputs back (weight 1.0 per selection).

Measured-on-HW design decisions (perfetto/NTFF trace driven):
  - bf16, not fp32r: the fp32r kernel was LDWEIGHTS-bound — a 128x128 fp32r
    stationary load runs ~226ns, longer than the matmul at NT=384 (~160ns),
    pinning the PE cadence at ~225ns/matmul. bf16 halves the weight load and
    all DMA traffic. End-to-end rel err 3.7e-3 vs the 2e-2 gate (routing
    stays exact: gate+top4 on host in fp32).
  - Not fp8: DoubleRow doubles the MAC rate, but raw e4m3 costs ~5% rel err
    (fails the gate) and error-compensated splits cost 3x the matmuls.
  - Token tiles are processed in PAIRS through each GEMM: the dc loop is
    outer, the tile loop inner, so consecutive matmuls share the stationary
    block and alternate PSUM banks (pools sized bufs=4). This removed a
    ~44ns/matmul bubble: cadence 236 -> 194ns at NT=461, i.e. ~1 col/cycle
    at 2.4GHz, Tensor HFU 0.92.
  - ~30 junk warmup matmuls on never-read SBUF/PSUM (no DMA deps) keep the
    Tensor engine busy from t~0: the DVFS p-state ramps to full clock while
    the startup DMA stream (w1s[0] + x0, ~2.4MB, ~17us) is in flight.
  - Startup DMA order is hand-tuned across both FIFO queues (scalar carries
    w1s[0] + half of x0 in parallel with sync). Weights stream in 128-wide
    column slices so GEMM chains start as their slice lands.
  - GEMM2 runs one tile-group behind GEMM1 (software pipeline) so the PE has
    GEMM1 work while w2 streams in; the last group is a single tile so the
    y-store drain after the final matmul is halved; late groups alternate
    their y stores across both DMA queues.

Per-core device layouts (everything pre-transposed on host for contiguous DMA):
  xT  [D, C] bf16 : routed tokens, transposed
  w1t [D, H] bf16 : w1[e].T
  w2t [H, O] bf16 : w2[e].T
  yT  [O, C] bf16 : expert output, transposed (host casts back + scatter-adds)

Capacity is exact (max expert load, even-rounded; 4146 for the seed-0 input
vs 4096 mean). SPMD requires identical per-core programs, so capacity
padding beats "perfect" 4096-balancing, which would need fixed 2-expert
segment slots totalling 4174.
"""

import numpy as np
import ml_dtypes
from contextlib import ExitStack

import concourse.bass as bass
import concourse.tile as tile
from concourse import bacc, mybir
from concourse import bass_utils

F32 = mybir.dt.float32
BF16 = mybir.dt.bfloat16
P = 128

TOP_K = 4
D, H, O, E = 2048, 1024, 2048, 8
_NC_CACHE = {}


def _tile_widths(C, target):
    """Split C tokens (padded to even) into even tiles of near-equal width in
    [256, 512]. 512 is the PSUM bank cap on a single matmul's moving dim.
    Uniform large tiles beat a small lead-in tile: the first tile's GEMM1 is
    the PE's only cover while the rest of w1/x/w2 stream in, so shrinking it
    just moves the idle later."""
    C = max(C + (C % 2), 256)
    if C > 1024:
        # last tile small: it alone sets the y-store drain after the final
        # matmul (a small FIRST tile is a measured regression — it shrinks
        # the PE's only cover while the rest of the startup stream lands)
        rest, last = C - 256, [256]
    else:
        rest, last = C, []
    C2 = rest // 2
    ntiles = min(-(-rest // target), C2 // 128)
    base = C2 // ntiles
    rem = C2 - base * ntiles
    widths = [2 * (base + 1)] * rem + [2 * base] * (ntiles - rem)
    widths.sort(reverse=True)
    widths += last
    assert sum(widths) == C and all(256 <= w <= 512 and w % 2 == 0 for w in widths)
    return widths


def build_expert_kernel(C, target):
    """Per-core program: dense [C, D] @ [D, H] -> relu -> @ [H, O] in bf16."""
    DC, HC, OC = D // P, H // P, O // P
    widths = _tile_widths(C, target)
    starts = [sum(widths[:i]) for i in range(len(widths))]
    NTILES = len(widths)
    NTMAX = max(widths)
    nc = bacc.Bacc("TRN2", target_bir_lowering=False, debug=False, num_devices=E)
    xT = nc.dram_tensor("xT", [D, C], BF16, kind="ExternalInput").ap()
    w1t = nc.dram_tensor("w1t", [D, H], BF16, kind="ExternalInput").ap()
    w2t = nc.dram_tensor("w2t", [H, O], BF16, kind="ExternalInput").ap()
    yT = nc.dram_tensor("yT", [O, C], BF16, kind="ExternalOutput").ap()

    with tile.TileContext(nc) as tc, ExitStack() as ctx:
        wpool = ctx.enter_context(tc.tile_pool(name="w", bufs=1))
        xpool = ctx.enter_context(tc.tile_pool(name="x", bufs=4))
        hpool = ctx.enter_context(tc.tile_pool(name="h", bufs=4))
        ypool = ctx.enter_context(tc.tile_pool(name="y", bufs=4))
        ps1 = ctx.enter_context(tc.tile_pool(name="ps1", bufs=4, space="PSUM"))
        ps2 = ctx.enter_context(tc.tile_pool(name="ps2", bufs=4, space="PSUM"))

        x_tiles = {}

        def dma_x(t, split=False):
            w_t = widths[t]
            x_t = xpool.tile([P, DC, NTMAX], BF16, name="x_t")[:, :, :w_t]
            src = xT[:, starts[t]:starts[t] + w_t].rearrange(
                "(dc p) n -> p dc n", p=P
            )
            if split:
                # quarter-sliced across both DMA queues: the first GEMM chain
                # reads dc slices in order, so with per-subtile deps its first
                # matmuls unblock after the first ~0.5MB instead of the full
                # tile; both queues issue in parallel
                q = DC // 4
                nc.sync.dma_start(x_t[:, :q, :], src[:, :q, :])
                nc.sync.dma_start(x_t[:, q:2 * q, :], src[:, q:2 * q, :])
                nc.scalar.dma_start(x_t[:, 2 * q:3 * q, :], src[:, 2 * q:3 * q, :])
                nc.scalar.dma_start(x_t[:, 3 * q:, :], src[:, 3 * q:, :])
            else:
                nc.sync.dma_start(x_t[:], src)
            x_tiles[t] = x_t

        # --- PE warmup: dependency-free junk matmuls (uninitialized SBUF,
        # result never read) keep the Tensor engine continuously busy from
        # t~0 so the DVFS p-state ramps to full clock while the startup DMA
        # stream is still in flight ---
        # raw SBUF tensors (not pool tiles): no init required, so the first
        # warmup matmul issues immediately at t~0 with no cross-engine
        # memset handshake; the junk values are never read
        wu_w = nc.alloc_sbuf_tensor("wu_w", [P, P], BF16).ap()
        wu_x = nc.alloc_sbuf_tensor("wu_x", [P, NTMAX], BF16).ap()
        wu_p = ps1.tile([P, NTMAX], F32, name="ph")
        NWU = 38
        for i in range(NWU):
            nc.tensor.matmul(wu_p[:], wu_w[:], wu_x[:], start=(i == 0),
                             stop=(i == NWU - 1))

        # --- startup DMA stream, hand-ordered across both FIFO queues:
        # scalar queue carries w1s[0] + half of x0, sync queue the other
        # half of x0 + the rest of w1 — the first GEMM chain unblocks after
        # ~1.4MB per queue issued in parallel ---
        w1s = []

        def dma_w1(hc, eng=None, halves=False):
            w = wpool.tile([P, DC, P], BF16, name=f"w1s{hc}")
            src = w1t[:, hc * P:(hc + 1) * P].rearrange("(dc p) h -> p dc h", p=P)
            e = eng or nc.sync
            if halves:
                # first half leads the scalar queue; second half trails the
                # sync queue so both startup queues carry ~1.2MB of the
                # first-chain critical set (reads wait on whole tiles, so
                # only the max over queues matters)
                e.dma_start(w[:, :DC // 2, :], src[:, :DC // 2, :])
                nc.sync.dma_start(w[:, DC // 2:, :], src[:, DC // 2:, :])
            else:
                e.dma_start(w[:], src)
            w1s.append(w)

        w2s = []

        def dma_w2(oc, eng=None):
            w = wpool.tile([P, HC, P], BF16, name=f"w2s{oc}")
            (eng or nc.sync).dma_start(
                w[:],
                w2t[:, oc * P:(oc + 1) * P].rearrange("(hc p) o -> p hc o", p=P),
            )
            w2s.append(w)

        dma_w1(0, eng=nc.scalar, halves=True)
        dma_x(0, split=True)
        for hc in range(1, HC):
            dma_w1(hc)
        if NTILES > 1:
            dma_x(1)
        if NTILES > 2:
            dma_x(2)
        for oc in range(OC):
            dma_w2(oc)

        def gemm1(ts):
            """Fused GEMM1 over a group of token tiles: the dc loop is outer,
            the tile loop inner, so consecutive matmuls share the stationary
            w1 block (amortizes the PE weight-swap bubble)."""
            hs = {}
            for t in ts:
                w_t = widths[t]
                hs[t] = hpool.tile([P, HC, NTMAX], BF16, name="h_t")[:, :, :w_t]
            phs = {}
            for hc in range(HC):
                for t in ts:
                    phs[t] = ps1.tile([P, NTMAX], F32, name="ph")[:, :widths[t]]
                for dc in range(DC):
                    for t in ts:
                        nc.tensor.matmul(
                            phs[t][:], w1s[hc][:, dc, :], x_tiles[t][:, dc, :],
                            start=(dc == 0), stop=(dc == DC - 1),
                        )
                for t in ts:
                    nc.scalar.activation(
                        hs[t][:, hc, :], phs[t][:],
                        mybir.ActivationFunctionType.Relu,
                    )
            for t in ts:
                x_tiles.pop(t)
                h_tiles[t] = hs[t]

        def gemm2(ts, late=False):
            """Fused GEMM2 over a group of token tiles (same-stationary).
            Late groups issue their y stores on the sync queue, which is idle
            once the input stream has finished, so the drain after the final
            matmul is split across two DMA queues."""
            hs = {t: h_tiles.pop(t) for t in ts}
            pos = {}
            for oc in range(OC):
                for t in ts:
                    pos[t] = ps2.tile([P, NTMAX], F32, name="po")[:, :widths[t]]
                for hc in range(HC):
                    for t in ts:
                        nc.tensor.matmul(
                            pos[t][:], w2s[oc][:, hc, :], hs[t][:, hc, :],
                            start=(hc == 0), stop=(hc == HC - 1),
                        )
                for t in ts:
                    w_t = widths[t]
                    y_t = ypool.tile([P, NTMAX], BF16, name="y_t")[:, :w_t]
                    nc.vector.tensor_copy(y_t[:], pos[t][:])
                    eng = nc.sync if late and oc % 2 == 0 else nc.scalar
                    eng.dma_start(
                        yT[oc * P:(oc + 1) * P, starts[t]:starts[t] + w_t],
                        y_t[:],
                    )

        # --- group tiles: tile 0 alone (starts as soon as x0+w1s[0] land),
        # the last tile alone (halves the y-store drain after the final
        # matmul), the rest in pairs; GEMM2 runs one group behind GEMM1 so
        # the PE has GEMM1 work while w2 streams in ---
        if NTILES == 1:
            groups = [(0,)]
        else:
            groups = (
                [(0,)]
                + [
                    tuple(range(t, min(t + 2, NTILES - 1)))
                    for t in range(1, NTILES - 1, 2)
                ]
                + [(NTILES - 1,)]
            )
        h_tiles = {}
        ngroups = len(groups)
        for gi, g in enumerate(groups):
            if gi + 1 < ngroups and gi >= 1:
                for t in groups[gi + 1]:
                    if t not in x_tiles:
                        dma_x(t)
            gemm1(g)
            if gi >= 1:
                gemm2(groups[gi - 1], late=(gi >= ngroups - 2))
        gemm2(groups[-1], late=True)
    nc.compile()
    return nc


def _route(xt, wg):
    """Host-side gate + top-4. Gap between 4th/5th gate values is ~3e-5 for
    this distribution, far above fp32 matmul noise, so fp32 reproduces the
    reference top-k set exactly."""
    gate = xt @ wg  # [N, E] fp32
    top4 = np.argpartition(-gate, TOP_K - 1, axis=1)[:, :TOP_K]  # set, unordered
    return top4


def kernel(x, wg, w1, w2, _want_results=False, _run_kwargs=None):
    x = np.asarray(x, dtype=np.float32)
    wg = np.asarray(wg, dtype=np.float32)
    w1 = np.asarray(w1, dtype=np.float32)
    w2 = np.asarray(w2, dtype=np.float32)
    B, S, Dx = x.shape
    N = B * S
    xt = np.ascontiguousarray(x.reshape(N, Dx))
    top4 = _route(xt, wg)

    # token lists per expert
    sel = np.zeros((N, E), dtype=bool)
    np.put_along_axis(sel, top4, True, axis=1)
    tokens = [np.nonzero(sel[:, e])[0] for e in range(E)]
    counts = np.array([len(t) for t in tokens])
    CAP = max(int(counts.max()), 256)
    CAP += CAP % 2

    if CAP not in _NC_CACHE:
        last_err = None
        for target in (512, 448, 384):
            try:
                _NC_CACHE[CAP] = build_expert_kernel(CAP, target)
                break
            except ValueError as err:  # SBUF pool allocation failure
                last_err = err
        else:
            raise last_err
    nc = _NC_CACHE[CAP]

    xtb = xt.astype(ml_dtypes.bfloat16)
    in_maps = []
    for e in range(E):
        xe = np.zeros((CAP, Dx), dtype=ml_dtypes.bfloat16)
        xe[:counts[e]] = xtb[tokens[e]]
        in_maps.append({
            "xT": np.ascontiguousarray(xe.T),
            "w1t": np.ascontiguousarray(w1[e].T.astype(ml_dtypes.bfloat16)),
            "w2t": np.ascontiguousarray(w2[e].T.astype(ml_dtypes.bfloat16)),
        })

    res = bass_utils.run_bass_kernel_spmd(
        nc, in_maps, core_ids=list(range(E)), **(_run_kwargs or {})
    )

    out = np.zeros((N, O), dtype=np.float32)
    for e in range(E):
        out[tokens[e]] += res.results[e]["yT"].T[:counts[e]].astype(np.float32)
    out = out.reshape(B, S, O)
    if _want_results:
        return out, res
    return out
